# revision 28
# baseline (speedup 1.0000x reference)
"""Trainium2 Bass kernel for CoordinationMemory (scatter_memory).

Computation (per batch row n):
    cur_h = memory[n, veh_idx[n], :]
    x     = concat(veh_repr[n], cust_repr[n], edge_emb[n])        # [3D]
    nh    = tanh(x @ W_in + b_in + cur_h @ W_h + b_h)             # [H]
    out   = memory with out[n, veh_idx[n], :] = nh

Full shapes: N=4096, L_V=64, H=512, D=256. Data-parallel over 8 cores
(512 rows each).

The output equals the input memory with only 512 of 32768 rows (per core)
replaced, so the kernel never copies the 64 MiB shard: the output DRAM
buffer is a donated input operand (the same mechanism run_bass_kernel_spmd's
axon path uses to pre-zero outputs, see bass2jax.run_bass_via_pjrt), and we
initialize it with the memory contents host-side.  On device each iteration
does the real work only:

  - one 512-row dma_gather (single SWDGE op; int16 indices in the wrapped
    [i%16, i//16] layout, replicated across the 8 GpSimd stripes),
  - the two GEMMs in bf16 (tolerance is 2e-2; bf16 lands at ~3e-3) with the
    gather-independent x @ W_in chunks issued first so the PE never stalls,
  - tanh on ACT, delta = next_h - cur_h on DVE,
  - one 512-row dma_scatter_add of the delta onto rows still holding cur_h,
    which yields exactly next_h (out[r] = cur_h[r] + (next_h - cur_h)).

A probe decomposition showed the previous 4-gather/4-scatter
indirect_dma_start chain cost 15.8 us/iter of fixed SWDGE op overhead
(~2 us per op, strictly FIFO on the one gpsimd queue); fusing each
direction into one 512-descriptor op removes most of that.

The bias and the x-GEMM are fused by augmenting x with a ones column
(padded to 896 = 7*128 contraction rows) and W_in with a b_in+b_h row.
"""

import time

import numpy as np

import concourse.bass as bass
import concourse.bass2jax as b2j
import concourse.tile as tile
from concourse import bacc, mybir
from concourse.masks import make_identity

N = 4096
LV = 64
H = 512
D = 256
NCORES = 8
NS = N // NCORES          # rows per core
KX = 896                  # padded x contraction dim: 768 data + 1 ones + pad
KXC = KX // 128           # 7 chunks
HC = H // 128              # 4 chunks
P = 128
NT = NS // P              # 128-row tiles per core

F32 = mybir.dt.float32
BF16 = mybir.dt.bfloat16
I32 = mybir.dt.int32


def build_program(
    ns=NS,
    lv=LV,
    h=H,
    kx=KX,
    repeats=1,
    gather_mode=None,  # "per_tile" | "one" | "gsa"; default from env
    unroll=False,
    order="xfirst",
    probe="full",  # "full" | "dma" | "compute" | "noscat"
):
    if gather_mode is None:
        import os

        gather_mode = os.environ.get("KERNEL_GATHER_MODE", "per_tile")
    nt = ns // P
    kxc = kx // P
    hc = h // P
    nc = bacc.Bacc(
        "TRN2",
        target_bir_lowering=False,
        debug=False,
        enable_asserts=False,
        num_devices=NCORES,
    )
    mem = nc.dram_tensor("mem", (ns, lv, h), F32, kind="ExternalInput").ap()
    xt = nc.dram_tensor("xt", (kxc, P, ns), BF16, kind="ExternalInput").ap()
    wtop = nc.dram_tensor("wtop", (kxc, P, h), BF16, kind="ExternalInput").ap()
    wh = nc.dram_tensor("wh", (hc, P, h), BF16, kind="ExternalInput").ap()
    # idx[p, t]  = row index within tile t's out tensor for batch row t*128+p
    # gidx[p, t] = row index within the full core shard (for the gather)
    idx = nc.dram_tensor("idx", (P, nt), I32, kind="ExternalInput").ap()
    gidx = nc.dram_tensor("gidx", (P, nt), I32, kind="ExternalInput").ap()
    single_out = gather_mode == "gsa"
    if single_out:
        # int16 whole-shard row indices in dma_gather's wrapped layout:
        # token i = batch row i lives at [i % 16, i // 16]
        gi16 = nc.dram_tensor("gi16", (P, ns // 16), mybir.dt.int16,
                              kind="ExternalInput").ap()
        out_single = nc.dram_tensor("out", (ns, lv, h), F32,
                                    kind="ExternalOutput").ap()
        outs = []
        out_flats = []
        out_single_flat = out_single.rearrange("n l h -> (n l) h")
    else:
        outs = [
            nc.dram_tensor(f"out{t}", (P, lv, h), F32, kind="ExternalOutput").ap()
            for t in range(nt)
        ]
        out_flats = [o.rearrange("n l h -> (n l) h") for o in outs]

    mem_flat = mem.rearrange("n l h -> (n l) h")

    with tile.TileContext(nc) as tc:
        with (
            tc.tile_pool(name="const", bufs=1) as constp,
            tc.tile_pool(name="gat", bufs=2) as gatp,
            tc.tile_pool(name="work", bufs=2) as workp,
            tc.tile_pool(name="stage", bufs=2) as stagep,
            tc.tile_pool(name="idxp", bufs=2) as idxp,
            tc.tile_pool(name="psum", bufs=2 if order == "orig" else 1, space="PSUM") as psump,
            tc.tile_pool(name="psumtr", bufs=2, space="PSUM") as psumtrp,
        ):
            ident = constp.tile([P, P], F32)
            make_identity(nc, ident[:])

            xt_sb = constp.tile([P, kxc * ns], BF16)
            for c in range(kxc):
                nc.scalar.dma_start(out=xt_sb[:, bass.ts(c, ns)], in_=xt[c])
            wtop_sb = constp.tile([P, kxc * h], BF16)
            for c in range(kxc):
                nc.scalar.dma_start(out=wtop_sb[:, bass.ts(c, h)], in_=wtop[c])
            wh_sb = constp.tile([P, hc * h], BF16)
            for c in range(hc):
                nc.scalar.dma_start(out=wh_sb[:, bass.ts(c, h)], in_=wh[c])

            scat_const = None
            if probe == "dma":
                scat_const = constp.tile([P, h], F32)
                for b in range(hc):
                    make_identity(nc, scat_const[:, bass.ts(b, P)])

            def body():
                if single_out:
                    gi16_sb = idxp.tile([P, ns // 16], mybir.dt.int16)
                    nc.scalar.dma_start(out=gi16_sb[:], in_=gi16[:])
                else:
                    idx_all = idxp.tile([P, nt], I32)
                    nc.scalar.dma_start(out=idx_all[:], in_=idx[:])
                    gidx_all = idxp.tile([P, nt], I32)
                    nc.scalar.dma_start(out=gidx_all[:], in_=gidx[:])

                # Gather all ns updated rows' current hidden state.
                cur_all = gatp.tile([P, nt * h], F32)
                if probe == "compute":
                    pass  # no gather: compute on whatever is in SBUF
                elif gather_mode == "gsa":
                    nc.gpsimd.dma_gather(
                        out_ap=cur_all[:].rearrange("p (t e) -> p t e", e=h),
                        in_ap=mem_flat[:],
                        idxs_ap=gi16_sb[:],
                        num_idxs=ns,
                        num_idxs_reg=ns,
                        elem_size=h,
                    )
                elif gather_mode == "one":
                    nc.gpsimd.indirect_dma_start(
                        out=cur_all[:],
                        out_offset=None,
                        in_=mem_flat[:],
                        in_offset=bass.IndirectOffsetOnAxis(ap=gidx_all[:], axis=0),
                    )
                else:
                    for t in range(nt):
                        nc.gpsimd.indirect_dma_start(
                            out=cur_all[:, bass.ts(t, h)],
                            out_offset=None,
                            in_=mem_flat[:],
                            in_offset=bass.IndirectOffsetOnAxis(
                                ap=gidx_all[:, t : t + 1], axis=0
                            ),
                        )

                def x_mms(t, pmm):
                    # gather-independent: the x @ W_in part of the GEMM
                    for c in range(kxc):
                        nc.tensor.matmul(
                            out=pmm[:],
                            lhsT=xt_sb[:, c * ns + t * P : c * ns + (t + 1) * P],
                            rhs=wtop_sb[:, bass.ts(c, h)],
                            start=(c == 0),
                            stop=False,
                        )

                def h_part(t, pmm):
                    # cur_h [n, h] -> cur_hT [h, n] in 128x128 blocks via PE,
                    # cast to bf16 in the PSUM->SBUF copy, then cur_h @ W_h.
                    cur_ht = workp.tile([P, h], BF16)
                    for b in range(hc):
                        ptr = psumtrp.tile([P, P], F32, space="PSUM")
                        nc.tensor.transpose(
                            out=ptr[:],
                            in_=cur_all[:, t * h + b * P : t * h + (b + 1) * P],
                            identity=ident[:],
                        )
                        nc.vector.tensor_copy(out=cur_ht[:, bass.ts(b, P)], in_=ptr[:])
                    for b in range(hc):
                        nc.tensor.matmul(
                            out=pmm[:],
                            lhsT=cur_ht[:, bass.ts(b, P)],
                            rhs=wh_sb[:, bass.ts(b, h)],
                            start=False,
                            stop=(b == hc - 1),
                        )

                def finish(t, pmm):
                    nh = stagep.tile([P, h], F32)
                    nc.scalar.activation(
                        out=nh[:],
                        in_=pmm[:],
                        func=mybir.ActivationFunctionType.Tanh,
                    )
                    nc.gpsimd.indirect_dma_start(
                        out=out_flats[t][:],
                        out_offset=bass.IndirectOffsetOnAxis(
                            ap=idx_all[:, t : t + 1], axis=0
                        ),
                        in_=nh[:],
                        in_offset=None,
                    )

                def finish_gsa(t, pmm, delta_all):
                    # tanh into the delta staging tile, then subtract cur_h
                    # in place: scatter-adding (next_h - cur_h) onto rows
                    # holding cur_h yields exactly next_h.
                    sl = slice(t * h, (t + 1) * h)
                    nc.scalar.activation(
                        out=delta_all[:, sl],
                        in_=pmm[:],
                        func=mybir.ActivationFunctionType.Tanh,
                    )
                    nc.vector.tensor_tensor(
                        out=delta_all[:, sl],
                        in0=delta_all[:, sl],
                        in1=cur_all[:, sl],
                        op=mybir.AluOpType.subtract,
                    )

                if probe == "dma":
                    # DMA-only: scatter a constant tile, no compute deps.
                    for t in range(nt):
                        nc.gpsimd.indirect_dma_start(
                            out=out_flats[t][:],
                            out_offset=bass.IndirectOffsetOnAxis(
                                ap=idx_all[:, t : t + 1], axis=0
                            ),
                            in_=scat_const[:],
                            in_offset=None,
                        )
                elif order == "xfirst":
                    # All gather-independent matmuls first so the PE never
                    # stalls on the gather; each tile gets its own PSUM bank.
                    pmms = [
                        psump.tile([P, h], F32, space="PSUM", name=f"pmm{t}")
                        for t in range(nt)
                    ]
                    for t in range(nt):
                        x_mms(t, pmms[t])
                    delta_all = (
                        stagep.tile([P, nt * h], F32, name="delta_all")
                        if single_out
                        else None
                    )
                    for t in range(nt):
                        h_part(t, pmms[t])
                        if single_out:
                            finish_gsa(t, pmms[t], delta_all)
                        elif probe != "noscat" and probe != "compute":
                            finish(t, pmms[t])
                        else:
                            nh = stagep.tile([P, h], F32, name=f"nhp{t}")
                            nc.scalar.activation(
                                out=nh[:],
                                in_=pmms[t][:],
                                func=mybir.ActivationFunctionType.Tanh,
                            )
                    if single_out:
                        nc.gpsimd.dma_scatter_add(
                            out_ap=out_single_flat[:],
                            in_ap=delta_all[:].rearrange("p (t e) -> p t e", e=h),
                            idxs_ap=gi16_sb[:],
                            num_idxs=ns,
                            num_idxs_reg=ns,
                            elem_size=h,
                        )
                else:
                    for t in range(nt):
                        pmm = psump.tile([P, h], F32, space="PSUM")
                        x_mms(t, pmm)
                        h_part(t, pmm)
                        finish(t, pmm)

            if repeats == 1:
                body()
            elif unroll:
                for _ in range(repeats):
                    body()
            else:
                with tc.For_i(0, repeats, 1):
                    body()

    nc.compile()
    return nc


class ProgramRunner:
    """Compile + execute a Bass program on the axon trn2 cores via PJRT.

    Mirrors the axon branch of bass_utils.run_bass_kernel_spmd
    (bass2jax.run_bass_via_pjrt), with two additions: inputs can be staged
    onto the devices once and reused across runs, and the donated
    ExternalOutput init buffers can carry caller data instead of zeros
    (elements the kernel doesn't write keep the init contents).
    """

    def __init__(self, nc, n_cores=NCORES):
        b2j.install_neuronx_cc_hook()
        self.nc = nc
        self.n_cores = n_cores
        pname = nc.partition_id_tensor.name if nc.partition_id_tensor else None
        self.dbg_name = nc.dbg_addr.name if nc.dbg_addr is not None else None
        in_names, out_names, out_avals = [], [], []
        import jax

        for alloc in nc.m.functions[0].allocations:
            if not isinstance(alloc, mybir.MemoryLocationSet):
                continue
            name = alloc.memorylocations[0].name
            if alloc.kind == "ExternalInput":
                if name != pname:
                    in_names.append(name)
            elif alloc.kind == "ExternalOutput":
                out_names.append(name)
                out_avals.append(
                    jax.core.ShapedArray(
                        tuple(alloc.tensor_shape), mybir.dt.np(alloc.dtype)
                    )
                )
        self.in_names = in_names
        self.out_names = out_names
        self.out_avals = out_avals
        all_in = in_names + out_names + ([pname] if pname else [])
        n_params, n_outs = len(in_names), len(out_names)
        donate = tuple(range(n_params, n_params + n_outs))

        def _body(*args):
            ops = list(args)
            if pname is not None:
                ops.append(b2j.partition_id_tensor())
            outs = b2j._bass_exec_p.bind(
                *ops,
                out_avals=tuple(out_avals),
                in_names=tuple(all_in),
                out_names=tuple(out_names),
                lowering_input_output_aliases=(),
                sim_require_finite=True,
                sim_require_nnan=True,
                nc=nc,
            )
            return tuple(outs)

        from jax.experimental.shard_map import shard_map
        from jax.sharding import Mesh, NamedSharding, PartitionSpec

        devices = jax.devices()[: n_cores]
        assert len(devices) == n_cores, (n_cores, jax.devices())
        self.mesh = Mesh(np.asarray(devices), ("core",))
        self.sharding = NamedSharding(self.mesh, PartitionSpec("core"))
        in_specs = (PartitionSpec("core"),) * (n_params + n_outs)
        out_specs = (PartitionSpec("core"),) * n_outs
        self.fn = jax.jit(
            shard_map(
                _body,
                mesh=self.mesh,
                in_specs=in_specs,
                out_specs=out_specs,
                check_rep=False,
            ),
            donate_argnums=donate,
            keep_unused=True,
        )
        self._jax = jax
        self.dev_inputs = None

    def put_inputs(self, in_maps):
        jax = self._jax
        arrs = []
        for name in self.in_names:
            if name == self.dbg_name and name not in in_maps[0]:
                per = [np.zeros((1, 2), np.uint32)] * self.n_cores
            else:
                per = [np.asarray(m[name]) for m in in_maps]
            arrs.append(jax.device_put(np.concatenate(per, axis=0), self.sharding))
        jax.block_until_ready(arrs)
        self.dev_inputs = arrs

    def _make_inits(self, out_inits):
        jax = self._jax
        inits = []
        for i, name in enumerate(self.out_names):
            aval = self.out_avals[i]
            if out_inits is None or name not in out_inits[0]:
                arr = np.zeros(
                    (self.n_cores * aval.shape[0], *aval.shape[1:]), aval.dtype
                )
            else:
                arr = np.concatenate(
                    [np.asarray(m[name]) for m in out_inits], axis=0
                )
            inits.append(jax.device_put(arr, self.sharding))
        jax.block_until_ready(inits)
        return inits

    def run_timed(self, out_inits=None):
        jax = self._jax
        inits = self._make_inits(out_inits)
        t0 = time.perf_counter()
        outs = self.fn(*self.dev_inputs, *inits)
        jax.block_until_ready(outs)
        t1 = time.perf_counter()
        return t1 - t0, outs

    def fetch(self, outs):
        res = [{} for _ in range(self.n_cores)]
        for i, name in enumerate(self.out_names):
            full = np.asarray(outs[i]).reshape(
                self.n_cores, *self.out_avals[i].shape
            )
            for c in range(self.n_cores):
                res[c][name] = full[c]
        return res


_RUNNER = None


def _get_runner():
    global _RUNNER
    if _RUNNER is None:
        _RUNNER = ProgramRunner(build_program())
    return _RUNNER


def make_in_maps(memory, veh_idx, veh_repr, cust_repr, edge_emb, W_in, b_in, W_h, b_h):
    bf = mybir.dt.np(BF16)
    memory = np.ascontiguousarray(np.asarray(memory, dtype=np.float32))
    veh_idx = np.asarray(veh_idx).astype(np.int64)
    x_cat = np.concatenate(
        (
            np.asarray(veh_repr, dtype=np.float32)[:, 0, :],
            np.asarray(cust_repr, dtype=np.float32)[:, 0, :],
            np.asarray(edge_emb, dtype=np.float32)[:, 0, 0, :],
            np.ones((N, 1), dtype=np.float32),
        ),
        axis=1,
    )  # [N, 769]

    wtop = np.zeros((KX, H), dtype=np.float32)
    wtop[: 3 * D] = np.asarray(W_in, dtype=np.float32)
    wtop[3 * D] = np.asarray(b_in, dtype=np.float32) + np.asarray(b_h, dtype=np.float32)
    wtop = np.ascontiguousarray(wtop.reshape(KXC, P, H)).astype(bf)
    wh = (
        np.ascontiguousarray(np.asarray(W_h, dtype=np.float32))
        .reshape(HC, P, H)
        .astype(bf)
    )

    in_maps = []
    out_inits = []
    for s in range(NCORES):
        lo, hi = s * NS, (s + 1) * NS
        xtf = np.zeros((KX, NS), dtype=np.float32)
        xtf[: 3 * D + 1] = x_cat[lo:hi].T
        # idx[p, t] = p*LV + veh_idx[t*128+p], relative to tile t's base;
        # gidx adds the tile base t*128*LV for the whole-shard gather.
        v = veh_idx[lo:hi, 0].reshape(NT, P).T
        idx = np.ascontiguousarray(
            (np.arange(P, dtype=np.int64)[:, None] * LV + v).astype(np.int32)
        )
        gidx = np.ascontiguousarray(
            idx + (np.arange(NT, dtype=np.int32)[None, :] * (P * LV))
        )
        # dma_gather/dma_scatter_add wrapped-int16 layout: token i at
        # [i%16, i//16], replicated across the 8 groups of 16 partitions
        # (each GpSimd core reads its own stripe).
        g_lin = (np.arange(NS, dtype=np.int64) * LV + v.T.reshape(-1)).astype(np.int16)
        gi16 = np.zeros((P, NS // 16), dtype=np.int16)
        for rep in range(P // 16):
            gi16[rep * 16 : rep * 16 + 16] = g_lin.reshape(NS // 16, 16).T
        in_maps.append(
            {
                "mem": memory[lo:hi],
                "xt": np.ascontiguousarray(xtf.reshape(KXC, P, NS)).astype(bf),
                "wtop": wtop,
                "wh": wh,
                "idx": idx,
                "gidx": gidx,
                "gi16": gi16,
            }
        )
        oi = {f"out{t}": memory[lo + t * P : lo + (t + 1) * P] for t in range(NT)}
        oi["out"] = memory[lo:hi]
        out_inits.append(oi)
    return in_maps, out_inits


def kernel(memory, veh_idx, veh_repr, cust_repr, edge_emb, W_in, b_in, W_h, b_h):
    runner = _get_runner()
    in_maps, out_inits = make_in_maps(
        memory, veh_idx, veh_repr, cust_repr, edge_emb, W_in, b_in, W_h, b_h
    )
    runner.put_inputs(in_maps)
    _, outs = runner.run_timed(out_inits)
    res = runner.fetch(outs)
    if "out" in res[0]:
        return np.concatenate([r["out"] for r in res], axis=0)
    return np.concatenate(
        [r[f"out{t}"] for r in res for t in range(NT)], axis=0
    )
